# revision 1
# baseline (speedup 1.0000x reference)
"""Trainium2 Bass kernel for nn_MultiModalTransformer (8-core data parallel).

Strategy: the reference network collapses to
    f   = relu(x0*we+be) ++ relu(x2*wb+bb)        # [B,32]; x1/img path is dead code
    h   = f @ Mcomb + chat                        # [B,64]
    y   = relu(bn(h)) @ F2c + f2_b                # [B,3], bn uses global batch stats
Batch is sharded 8 ways. BatchNorm statistics are computed WITHOUT touching h:
    sum(h)  comes from  SF = sum(f)        (via Mcomb)
    sum(h²) comes from  G  = f^T f (Gram)  (via Mcomb^T G Mcomb diag)
G/SF partials are AllReduced across cores (tiny ~4KB payload), then each core
applies BN+relu+f2 in a second pass over on-chip cached f.

All layout changes (x is [B,3] interleaved; output [B,3]) are PE transposes +
wide SBUF->SBUF DMA gathers -- never small strided DMA.

Sample indexing per core (Bs = 128*NB):  u = p*NB + 32*c0 + 4*g + a
f4 column w = c0*1024 + g*128 + p, partition row = 32*a + j (4 samples/column).
"""
import numpy as np

E = 16
EPS = 1e-5
P = 128
N_CORES = 8
B_FULL = 1048576
BS = B_FULL // N_CORES          # 131072 per core

_CACHE = {}


def _derive_host_constants(w):
    dt = np.float64
    g = {k: np.asarray(v, dt) for k, v in w.items()}
    Wv_b, bv_b = g["bio_qkv_w"][2 * E:3 * E], g["bio_qkv_b"][2 * E:3 * E]
    Wv_e, bv_e = g["ehr_qkv_w"][2 * E:3 * E], g["ehr_qkv_b"][2 * E:3 * E]
    Watt, batt = g["attn_in_w"][2 * E:3 * E], g["attn_in_b"][2 * E:3 * E]
    Wout, bout = g["attn_out_w"], g["attn_out_b"]

    M_mha = Watt.T @ Wout.T
    c_mha = batt @ Wout.T + bout
    P1 = g["ab_proj_w"][:, :E]
    P2 = g["ab_proj_w"][:, E:]
    A_e = Wv_e.T @ M_mha @ P1.T
    A_b = Wv_b.T @ M_mha @ P2.T
    c_ab = (bv_e @ M_mha + c_mha) @ P1.T + (bv_b @ M_mha + c_mha) @ P2.T + g["ab_proj_b"]
    F1a = g["f1_w"][:, :E].T
    F1b = g["f1_w"][:, E:].T
    Me = A_e @ F1a + A_e @ M_mha @ F1b
    Mb = A_b @ F1a + A_b @ M_mha @ F1b
    chat = c_ab @ F1a + (c_ab @ M_mha + c_mha) @ F1b + g["f1_b"]
    Mcomb = np.concatenate([Me, Mb], axis=0)      # [32,64]

    we, be = g["ehr_w"][:, 0], g["ehr_b"]
    wb, bb = g["bio_w"][:, 0], g["bio_b"]

    lhsT_w = np.zeros((8, 128), dt)
    bias4 = np.zeros((128, 1), dt)
    for a in range(4):
        lhsT_w[2 * a + 0, 32 * a + np.arange(16)] = we
        lhsT_w[2 * a + 1, 32 * a + 16 + np.arange(16)] = wb
        bias4[32 * a + np.arange(16), 0] = be
        bias4[32 * a + 16 + np.arange(16), 0] = bb

    Mh = np.zeros((128, 128), dt)     # two stacked copies: s-half at base partition 64*s
    for hh in range(2):
        Mh[64 * hh + 0:64 * hh + 32, 0:64] = Mcomb
        Mh[64 * hh + 32:64 * hh + 64, 64:128] = Mcomb

    lhsT_fold = np.zeros((128, 32), dt)
    for a in range(4):
        lhsT_fold[32 * a + np.arange(32), np.arange(32)] = 1.0

    f2bias = np.zeros((128, 1), dt)
    for t in range(4):
        for sg in range(2):
            f2bias[32 * t + 3 * sg:32 * t + 3 * sg + 3, 0] = g["f2_b"]

    import ml_dtypes
    f32 = np.float32
    bf16 = ml_dtypes.bfloat16
    consts = dict(
        lhsT_w=lhsT_w.astype(bf16),
        bias4=bias4.astype(f32),
        Mh=Mh.astype(bf16),
        Mcomb=Mcomb.astype(f32),
        chat_col=chat.astype(f32).reshape(64, 1),
        ones32=np.ones((32, 1), f32),
        lhsT_fold=lhsT_fold.astype(f32),
        F2c=g["f2_w"].T.astype(f32),              # [64,3]
        g_col=g["bn_g"].astype(f32).reshape(64, 1),
        b_col=g["bn_b"].astype(f32).reshape(64, 1),
        f2bias=f2bias.astype(f32),
        ident=np.eye(128, dtype=f32),
        ident_bf=np.eye(128).astype(bf16),
        eps_col=np.full((64, 1), EPS, f32),
        bog_col=(np.where(g["bn_g"] != 0, g["bn_b"] / np.where(g["bn_g"] == 0, 1, g["bn_g"]), 0.0)
                 ).astype(f32).reshape(64, 1),
    )
    return consts


def _build(Bs, n_cores, bn_frac_act=9, collective=True, fast_bn=True):
    """Build+schedule the Bass program. bn_frac_act: of 20 BN ops, how many on ACT."""
    from concourse import bacc, mybir, tile

    f32 = mybir.dt.float32
    NB = Bs // P                 # samples per partition
    NCH = NB // 32               # 96-col transpose chunks; also pass-2 chunks
    Wtot = Bs // 4               # f4 columns
    GRP = 4                      # c0 chunks per pass-1 group
    assert NCH % GRP == 0
    YB = min(8, NCH)             # c0 chunks per output-DMA batch
    assert NCH % YB == 0

    nc = bacc.Bacc("TRN2", target_bir_lowering=False, debug=False,
                   num_devices=n_cores)
    x = nc.dram_tensor("x", [Bs, 3], f32, kind="ExternalInput")
    y = nc.dram_tensor("y", [Bs, 3], f32, kind="ExternalOutput")

    cshapes = dict(lhsT_w=[8, 128], bias4=[128, 1], Mh=[128, 128],
                   Mcomb=[32, 64], chat_col=[64, 1], ones32=[32, 1],
                   lhsT_fold=[128, 32], F2c=[64, 3], g_col=[64, 1],
                   b_col=[64, 1], f2bias=[128, 1], ident=[128, 128],
                   eps_col=[64, 1], bog_col=[64, 1], ident_bf=[128, 128])
    bf = mybir.dt.bfloat16
    BF16_CONSTS = {"lhsT_w", "Mh", "ident_bf"}
    cdram = {k: nc.dram_tensor(k, s, bf if k in BF16_CONSTS else f32,
                               kind="ExternalInput")
             for k, s in cshapes.items()}

    AF = mybir.ActivationFunctionType
    OP = mybir.AluOpType
    AX = mybir.AxisListType

    with tile.TileContext(nc) as tc:
        from contextlib import ExitStack
        ctx = ExitStack()
        with ctx:
            consts = ctx.enter_context(tc.tile_pool(name="consts", bufs=1))
            C = {}
            for k, s in cshapes.items():
                C[k] = consts.tile(s, bf if k in BF16_CONSTS else f32,
                                   tag=k, name=f"c_{k}")
                nc.sync.dma_start(C[k][:], cdram[k].ap())

            # ACT table preload (sqrt set includes relu/identity/copy/square)
            warm = consts.tile([64, 1], f32, tag="warm")
            nc.scalar.activation(warm[:], C["chat_col"][:], AF.Sqrt,
                                 bias=C["eps_col"][:], scale=0.0)

            f4pool = ctx.enter_context(tc.tile_pool(name="f4", bufs=1))
            f4 = f4pool.tile([128, Wtot], bf, tag="f4")

            sfpool = ctx.enter_context(tc.tile_pool(name="sf", bufs=1))
            sf_cols = sfpool.tile([128, 2 * NCH], f32, tag="sfc")
            n_sf = 0

            # ---------------- PASS 1: x -> f4 -> gram (fused) ----------------
            import contextlib
            gram_scope = contextlib.ExitStack()
            gramps = gram_scope.enter_context(
                tc.tile_pool(name="p1_gram", bufs=1, space="PSUM"))
            gram = gramps.tile([128, 128], f32, tag="gram")
            n_sl = Wtot // 128
            with tc.tile_pool(name="p1a_sb", bufs=6) as p1sb, \
                 tc.tile_pool(name="p1b_sb", bufs=6) as p2sb, \
                 tc.tile_pool(name="p1a_x", bufs=1) as pxsb, \
                 tc.tile_pool(name="p1a_ps", bufs=1, space="PSUM") as p1ps, \
                 tc.tile_pool(name="p1b_ps", bufs=2, space="PSUM") as p2ps, \
                 tc.tile_pool(name="p1a_ps2", bufs=2, space="PSUM") as p1ps2:
                x_sb = pxsb.tile([128, 3 * NB], f32, tag="x_sb")
                xsrc = x.ap().rearrange("(p i) c -> p (i c)", p=P)
                CH = 3 * NB // (NCH // GRP)
                for ld in range(NCH // GRP):
                    nc.sync.dma_start(x_sb[:, CH * ld:CH * (ld + 1)],
                                      xsrc[:, CH * ld:CH * (ld + 1)])

                for grp in range(NCH // GRP):
                    # permute cols to (c,a,g) order so the transpose rows come
                    # out as r' = c*32 + a*8 + g -> gathers are contiguous blocks
                    xpm = p1sb.tile([128, 96 * GRP], bf, tag="xpm")
                    nc.vector.tensor_copy(
                        xpm[:],
                        x_sb[:, 96 * GRP * grp:96 * GRP * (grp + 1)].rearrange(
                            "p (cc g a c) -> p cc c a g", cc=GRP, g=8, a=4, c=3))
                    xT_ps = p1ps.tile([96, 128 * GRP], bf, tag="xT")
                    for i in range(GRP):
                        nc.tensor.transpose(
                            xT_ps[:, 128 * i:128 * (i + 1)],
                            xpm[:, 96 * i:96 * (i + 1)], C["ident_bf"][:])
                    xTs = p1sb.tile([96, 128 * GRP], bf, tag="xTs")
                    nc.vector.tensor_copy(xTs[:], xT_ps[:])
                    xp = p1sb.tile([8, 1024 * GRP], bf, tag="xp")
                    # xp col = g*(128*GRP) + c*128 + p  (g-major; plain DMA)
                    for a in range(4):
                        for comp in range(2):
                            r0 = 64 * comp + 8 * a
                            nc.sync.dma_start(
                                xp[2 * a + comp:2 * a + comp + 1, :],
                                xTs[r0:r0 + 8, :])
                    # f_pre matmuls: two 512-col windows per 2-bank psum tile
                    for half in range(GRP):
                        fp_ps = p1ps2.tile([128, 1024], f32, tag="fpre")
                        for q in range(2):
                            wloc = half * 1024 + q * 512
                            nc.tensor.matmul(
                                fp_ps[:, 512 * q:512 * (q + 1)],
                                C["lhsT_w"][:], xp[:, wloc:wloc + 512],
                                start=True, stop=True)
                        wglob = grp * 4096 + half * 1024
                        nc.scalar.activation(
                            f4[:, wglob:wglob + 1024], fp_ps[:],
                            AF.Relu, bias=C["bias4"][:],
                            accum_out=sf_cols[:, n_sf:n_sf + 1])
                        n_sf += 1
                    # ---- gram sub-phase for this group's f4 region ----
                    for S4 in range(GRP * 2):
                        ft_ps = p2ps.tile([128, 512], bf, tag="ftps")
                        for i in range(4):
                            S = grp * (8 * GRP) + S4 * 4 + i
                            nc.tensor.transpose(
                                ft_ps[:, 128 * i:128 * (i + 1)],
                                f4[:, 128 * S:128 * (S + 1)], C["ident_bf"][:])
                        ft_sb = p2sb.tile([128, 512], bf, tag="ftsb")
                        if S4 % 2 == 0:
                            nc.vector.tensor_copy(ft_sb[:], ft_ps[:])
                        else:
                            nc.scalar.copy(ft_sb[:], ft_ps[:])
                        for i in range(4):
                            S = grp * (8 * GRP) + S4 * 4 + i
                            nc.tensor.matmul(
                                gram[:], ft_sb[:, 128 * i:128 * (i + 1)],
                                ft_sb[:, 128 * i:128 * (i + 1)],
                                start=(S == 0), stop=(S == n_sl - 1))

            # gram halves fold (pass-1b was merged into the group loop)
            statpool = ctx.enter_context(tc.tile_pool(name="stat", bufs=1))
            with gram_scope:
                gram_sb = statpool.tile([128, 128], f32, tag="gram_sb")
                nc.vector.tensor_copy(gram_sb[:], gram[:])

            with tc.tile_pool(name="mid_sb", bufs=1) as midsb, \
                 tc.tile_pool(name="mid_ps", bufs=1, space="PSUM") as midps, \
                 tc.tile_pool(name="mid_dram", bufs=1, space="DRAM") as middr:
                Gcols = midsb.tile([32, 128], f32, tag="Gcols")
                for a in range(4):
                    nc.sync.dma_start(
                        Gcols[0:32, 32 * a:32 * a + 32],
                        gram_sb[32 * a:32 * a + 32, 32 * a:32 * a + 32])
                pay = midsb.tile([32, 33], f32, tag="pay")
                nc.vector.tensor_reduce(
                    pay[:, 0:32],
                    Gcols[:].rearrange("p (a j) -> p j a", a=4),
                    axis=AX.X, op=OP.add)
                sf_fin = midsb.tile([128, 1], f32, tag="sf_fin")
                nc.vector.tensor_reduce(
                    sf_fin[:], sf_cols[:, 0:n_sf], axis=AX.X, op=OP.add)
                sf32_ps = midps.tile([32, 2], f32, tag="sf32ps")
                nc.tensor.matmul(sf32_ps[:, 0:1], C["lhsT_fold"][:], sf_fin[:],
                                 start=True, stop=True)
                nc.scalar.copy(pay[:, 32:33], sf32_ps[:, 0:1])

                ar_in = middr.tile([32, 33], f32)
                ar_out = middr.tile([32, 33], f32, addr_space="Shared")
                nc.sync.dma_start(ar_in[:], pay[:])
                if collective:
                    nc.gpsimd.collective_compute(
                        "AllReduce", OP.add,
                        replica_groups=[list(range(n_cores))],
                        ins=[ar_in.opt()], outs=[ar_out.opt()])
                else:
                    nc.gpsimd.dma_start(ar_out[:], ar_in[:])
                payg = midsb.tile([32, 33], f32, tag="payg")
                nc.sync.dma_start(payg[:], ar_out[:])
                G32 = payg[:, 0:32]
                SF32 = payg[:, 32:33]

                # ---- stats math on [64,1] vectors ----
                Bf = float(Bs * n_cores)
                mt_ps = midps.tile([64, 2], f32, tag="mtps")
                nc.tensor.matmul(mt_ps[:, 0:1], C["Mcomb"][:], SF32,
                                 start=True, stop=True)
                v1_ps = midps.tile([32, 64], f32, tag="v1ps")
                nc.tensor.matmul(v1_ps[:], G32, C["Mcomb"][:],
                                 start=True, stop=True)
                w1 = midsb.tile([32, 64], f32, tag="w1")
                nc.vector.tensor_mul(w1[:], v1_ps[:], C["Mcomb"][:])
                mgm_ps = midps.tile([64, 2], f32, tag="mgmps")
                nc.tensor.matmul(mgm_ps[:, 0:1], w1[:], C["ones32"][:],
                                 start=True, stop=True)

                sv = midsb.tile([64, 10], f32, tag="sv")
                q_ = sv[:, 0:1]; mu = sv[:, 1:2]; t2 = sv[:, 2:3]
                t3 = sv[:, 3:4]; e2 = sv[:, 4:5]; var = sv[:, 5:6]
                sd = sv[:, 6:7]; inv = sv[:, 7:8]; s_c = sv[:, 8:9]
                bi = sv[:, 9:10]
                nc.scalar.activation(q_, mt_ps[:, 0:1], AF.Copy, scale=1.0 / Bf)
                nc.scalar.activation(mu, mt_ps[:, 0:1], AF.Identity,
                                     bias=C["chat_col"][:], scale=1.0 / Bf)
                nc.vector.tensor_add(t2, q_, mu)
                nc.vector.tensor_mul(t3, C["chat_col"][:], t2)
                nc.scalar.activation(e2, mgm_ps[:, 0:1], AF.Identity,
                                     bias=t3, scale=1.0 / Bf)
                nc.vector.tensor_mul(var, mu, mu)
                nc.vector.tensor_sub(var, e2, var)
                nc.scalar.activation(sd, var, AF.Sqrt, bias=C["eps_col"][:])
                nc.vector.reciprocal(inv, sd)
                nc.vector.tensor_mul(s_c, C["g_col"][:], inv)
                # bias = s*(chat - mu) + b
                nc.vector.tensor_sub(bi, C["chat_col"][:], mu)
                nc.vector.tensor_mul(bi, s_c, bi)
                nc.vector.tensor_add(bi, bi, C["b_col"][:])

                s_rep = statpool.tile([128, 1], f32, tag="s_rep")
                b_rep = statpool.tile([128, 1], f32, tag="b_rep")
                f2sb = statpool.tile([128, 32], bf, tag="f2sb")
                nc.vector.memset(f2sb[:], 0.0)
                if fast_bn:
                    # rh = relu(h + bos), bos = (chat-mu) + (b/g)*sd ; s folded
                    # into the f2 weights so scale never touches the hot loop
                    bos = sv[:, 2:3]       # reuse t2 slot (dead by now)
                    nc.vector.tensor_sub(bos, C["chat_col"][:], mu)
                    t7 = sv[:, 3:4]
                    nc.vector.tensor_mul(t7, C["bog_col"][:], sd)
                    nc.vector.tensor_add(bos, bos, t7)
                    F2cs = midsb.tile([64, 3], bf, tag="F2cs")
                    nc.vector.tensor_scalar(F2cs[:], C["F2c"][:], s_c, None,
                                            op0=OP.mult)
                    for hh in range(2):
                        nc.sync.dma_start(b_rep[64 * hh:64 * hh + 64, :], bos)
                        nc.sync.dma_start(
                            f2sb[64 * hh:64 * hh + 64, 3 * hh:3 * hh + 3],
                            F2cs[:])
                else:
                    F2cb = midsb.tile([64, 3], bf, tag="F2cb")
                    nc.vector.tensor_copy(F2cb[:], C["F2c"][:])
                    for hh in range(2):
                        nc.sync.dma_start(s_rep[64 * hh:64 * hh + 64, :], s_c)
                        nc.sync.dma_start(b_rep[64 * hh:64 * hh + 64, :], bi)
                        nc.sync.dma_start(
                            f2sb[64 * hh:64 * hh + 64, 3 * hh:3 * hh + 3],
                            F2cb[:])

            # ---------------- PASS 2 ----------------
            with tc.tile_pool(name="p3_sb", bufs=6) as p3sb, \
                 tc.tile_pool(name="p3_yc", bufs=4) as p3yc, \
                 tc.tile_pool(name="p3_hps", bufs=3, space="PSUM") as hpsp, \
                 tc.tile_pool(name="p3_yps", bufs=1, space="PSUM") as ypsp, \
                 tc.tile_pool(name="p3_tps", bufs=1, space="PSUM") as tpsp:
                n_bn = 0
                for grp in range(NCH // GRP):
                    yc = p3yc.tile([128, 96 * GRP], f32, tag="yc")
                    for jj in range(4):
                        py_ps = ypsp.tile([128, 512], f32, tag="pyps")
                        for wi in range(2):
                            wt = grp * 8 + 2 * jj + wi
                            h_ps = hpsp.tile([128, 1024], f32, tag="hps")
                            for s in range(2):
                                nc.tensor.matmul(
                                    h_ps[:, 512 * s:512 * (s + 1)],
                                    C["Mh"][64 * s:64 * (s + 1), :],
                                    f4[64 * s:64 * (s + 1), 512 * wt:512 * (wt + 1)],
                                    start=True, stop=True)
                            rh = p3sb.tile([128, 1024], bf, tag="rh")
                            if fast_bn:
                                if n_bn % 20 < bn_frac_act:
                                    nc.scalar.activation(rh[:], h_ps[:], AF.Relu,
                                                         bias=b_rep[:])
                                else:
                                    nc.vector.tensor_scalar(
                                        rh[:], h_ps[:], b_rep[:], 0.0,
                                        op0=OP.add, op1=OP.max)
                            else:
                                nc.scalar.activation(rh[:], h_ps[:], AF.Relu,
                                                     bias=b_rep[:], scale=s_rep[:])
                            n_bn += 1
                            for s in range(2):
                                t = 2 * wi + s
                                nc.tensor.matmul(
                                    py_ps[32 * t:32 * t + 32, :], f2sb[:],
                                    rh[:, 512 * s:512 * (s + 1)],
                                    start=True, stop=True,
                                    tile_position=(0, 32 * t))
                        ystage = p3sb.tile([128, 512], f32, tag="ystage")
                        nc.scalar.activation(ystage[:], py_ps[:], AF.Identity,
                                             bias=C["f2bias"][:])
                        yT_ps = tpsp.tile([128, 512], f32, tag="ytps")
                        for k in range(4):
                            nc.tensor.transpose(
                                yT_ps[:, 128 * k:128 * (k + 1)],
                                ystage[:, 128 * k:128 * (k + 1)],
                                C["ident"][:])
                        # yc col = (32c + 8j + 4wi + 2s + sg)*3 + m
                        ycv = yc[:].rearrange(
                            "p (k j w s r) -> p j w k s r", k=4, j=4, w=2, s=2, r=6)
                        ytv = yT_ps[:].rearrange(
                            "p (k w s r) -> p w k s r", k=4, w=2, s=2, r=32)
                        for wi in range(2):
                            nc.vector.tensor_copy(
                                ycv[:, jj, wi], ytv[:, wi, :, :, 0:6])
                    dsty = y.ap().rearrange(
                        "(p g b) m -> p g (b m)", p=P, b=128)[:, grp, :]
                    nc.sync.dma_start(dsty, yc[:])

    nc.compile()
    return nc


def _get_nc(Bs, n_cores, fast_bn):
    key = (Bs, n_cores, fast_bn)
    if key not in _CACHE:
        _CACHE[key] = _build(Bs, n_cores, fast_bn=fast_bn)
    return _CACHE[key]


def kernel(**inputs):
    from concourse.bass_utils import run_bass_kernel_spmd

    x_full = np.ascontiguousarray(np.asarray(inputs["x"], np.float32))
    B = x_full.shape[0]
    assert B % N_CORES == 0
    Bs = B // N_CORES
    w = {k: np.asarray(v) for k, v in inputs.items() if k != "x"}
    consts = _derive_host_constants(w)

    fast_bn = bool(np.all(np.asarray(w["bn_g"]) > 0))
    nc = _get_nc(Bs, N_CORES, fast_bn)
    in_maps = []
    for c in range(N_CORES):
        m = {"x": np.ascontiguousarray(x_full[c * Bs:(c + 1) * Bs])}
        m.update(consts)
        in_maps.append(m)
    res = run_bass_kernel_spmd(nc, in_maps, core_ids=list(range(N_CORES)))
    return np.concatenate([res.results[c]["y"] for c in range(N_CORES)], axis=0)



# revision 23
# speedup vs baseline: 1.2888x; 1.2888x over previous
"""Trainium2 Bass kernel for nn_MultiModalTransformer (8-core data parallel).

Strategy: the reference network collapses to
    f   = relu(x0*we+be) ++ relu(x2*wb+bb)        # [B,32]; x1/img path is dead code
    h   = f @ Mcomb + chat                        # [B,64]
    y   = relu(bn(h)) @ F2c + f2_b                # [B,3], bn uses global batch stats
Batch is sharded 8 ways. BatchNorm statistics are computed WITHOUT touching h:
    sum(h)  comes from  SF = sum(f)        (via Mcomb)
    sum(h^2) comes from  G  = f^T f (Gram)  (via Mcomb^T G Mcomb diag)
G/SF partials are AllReduced across cores (tiny ~4KB payload), then each core
applies BN+relu+f2 in a second pass over on-chip cached f.

v2 layout highlights (vs the earlier PE-transpose-heavy version):
  * f^T for the Gram is produced by the DMA XBAR transpose engine
    (dma_start transpose=True, contiguous 3D [128, nblk, 128] dest), so the
    PE only runs the Gram matmuls themselves and the 256 PE transposes +
    64 PSUM->SBUF copies are gone.
  * Gram+rowsum per 128-col block: one lhsT load + 5 matmuls into a single
    [128, 133] PSUM tile (4 alpha-slices of G cross blocks + a ones-column
    for SF); diag blocks extracted in the tiny mid-section.
  * The final y transpose is also a DMA XBAR transpose of a bf16 staging
    buffer; f2 output rows are placed at 16*sg+m so the post-transpose
    gather APs stay affine.
  * BN relu / f4 relu / ystage bias work is split between ACT and DVE;
    the yc gather copies run mostly on GPSIMD (SBUF-only engine).

Sample indexing per core (Bs = 128*NB):  u = p*NB + i,  i = 32*c0 + 4*g + a,
c0 = 8*G + 2*beta + cci.  f4 column (within group G) w = v*512 + beta*128 + p
with v = 8*cci + g; f4 partition row = 32*a + j.
"""
import numpy as np

E = 16
EPS = 1e-5
P = 128
N_CORES = 8
B_FULL = 1048576
BS = B_FULL // N_CORES          # 131072 per core

_CACHE = {}


def _derive_host_constants(w):
    dt = np.float64
    g = {k: np.asarray(v, dt) for k, v in w.items()}
    Wv_b, bv_b = g["bio_qkv_w"][2 * E:3 * E], g["bio_qkv_b"][2 * E:3 * E]
    Wv_e, bv_e = g["ehr_qkv_w"][2 * E:3 * E], g["ehr_qkv_b"][2 * E:3 * E]
    Watt, batt = g["attn_in_w"][2 * E:3 * E], g["attn_in_b"][2 * E:3 * E]
    Wout, bout = g["attn_out_w"], g["attn_out_b"]

    M_mha = Watt.T @ Wout.T
    c_mha = batt @ Wout.T + bout
    P1 = g["ab_proj_w"][:, :E]
    P2 = g["ab_proj_w"][:, E:]
    A_e = Wv_e.T @ M_mha @ P1.T
    A_b = Wv_b.T @ M_mha @ P2.T
    c_ab = (bv_e @ M_mha + c_mha) @ P1.T + (bv_b @ M_mha + c_mha) @ P2.T + g["ab_proj_b"]
    F1a = g["f1_w"][:, :E].T
    F1b = g["f1_w"][:, E:].T
    Me = A_e @ F1a + A_e @ M_mha @ F1b
    Mb = A_b @ F1a + A_b @ M_mha @ F1b
    chat = c_ab @ F1a + (c_ab @ M_mha + c_mha) @ F1b + g["f1_b"]
    Mcomb = np.concatenate([Me, Mb], axis=0)      # [32,64]

    we, be = g["ehr_w"][:, 0], g["ehr_b"]
    wb, bb = g["bio_w"][:, 0], g["bio_b"]

    # f_pre lhsT: row q' = 4*cp + a -> outputs 32*a + 16*cp + j
    lhsT_w = np.zeros((8, 128), dt)
    bias4 = np.zeros((128, 1), dt)
    for a in range(4):
        lhsT_w[a, 32 * a + np.arange(16)] = we
        lhsT_w[4 + a, 32 * a + 16 + np.arange(16)] = wb
        bias4[32 * a + np.arange(16), 0] = be
        bias4[32 * a + 16 + np.arange(16), 0] = bb

    Mh = np.zeros((128, 128), dt)     # two stacked copies: sg-half at base partition 32*sg
    for hh in range(2):
        Mh[64 * hh + 0:64 * hh + 32, 0:64] = Mcomb
        Mh[64 * hh + 32:64 * hh + 64, 64:128] = Mcomb

    lhsT_fold = np.zeros((128, 32), dt)
    for a in range(4):
        lhsT_fold[32 * a + np.arange(32), np.arange(32)] = 1.0

    # f2 bias replicated at rows 32*t + 16*sg + m
    f2bias = np.zeros((128, 1), dt)
    for t in range(4):
        for sg in range(2):
            f2bias[32 * t + 16 * sg:32 * t + 16 * sg + 3, 0] = g["f2_b"]

    import ml_dtypes
    f32 = np.float32
    bf16 = ml_dtypes.bfloat16
    rep2 = lambda v: np.concatenate([v, v]).reshape(128, 1)
    bog = np.where(g["bn_g"] != 0, g["bn_b"] / np.where(g["bn_g"] == 0, 1, g["bn_g"]), 0.0)
    consts = dict(
        lhsT_w=lhsT_w.astype(bf16),
        bias4=bias4.astype(f32),
        Mh=Mh.astype(bf16),
        Mcomb2=np.concatenate([Mcomb, Mcomb], axis=1).astype(f32),  # [32,128]
        chat2=rep2(chat).astype(f32),
        ones32=np.ones((32, 1), f32),
        ones_bf=np.ones((128, 1)).astype(bf16),
        lhsT_fold=lhsT_fold.astype(f32),
        F2c2=np.concatenate([g["f2_w"].T, g["f2_w"].T]).astype(f32),  # [128,3]
        g2=rep2(g["bn_g"]).astype(f32),
        b2=rep2(g["bn_b"]).astype(f32),
        f2bias=f2bias.astype(f32),
        ident_bf=np.eye(128).astype(bf16),
        eps2=np.full((128, 1), EPS, f32),
        bog2=rep2(bog).astype(f32),
    )
    return consts


def _bres(i, num, den):
    """Bresenham engine picker: True for `num` of every `den`, interleaved."""
    return (i * num) % den < num


def _build(Bs, n_cores, collective=True, fast_bn=True,
           p1_act=22, rh_act=34, yst_act=24, yc_pool=3):
    """Build+schedule the Bass program.

    p1_act: of 32 pass-1 f4 tiles, how many on ACT (rest DVE).
    rh_act: of 64 pass-2 rh tiles, how many on ACT (rest DVE).
    yst_act: of 32 ystage tiles, how many on ACT (rest DVE).
    yc_pool: of every 4 yc copies, how many on GPSIMD (rest DVE).
    """
    from concourse import bacc, mybir, tile

    f32 = mybir.dt.float32
    bf = mybir.dt.bfloat16
    NB = Bs // P                 # samples per partition (1024)
    NCH = NB // 32               # c0 chunks (32)
    GRP = 8                      # c0 chunks per group
    assert NCH % GRP == 0
    NG = NCH // GRP              # groups (4)
    WG = GRP * 1024              # f4 cols per group (8192)
    Wtot = NG * WG
    assert Wtot == Bs // 4

    nc = bacc.Bacc("TRN2", target_bir_lowering=False, debug=False,
                   num_devices=n_cores)
    x = nc.dram_tensor("x", [Bs, 3], f32, kind="ExternalInput")
    y = nc.dram_tensor("y", [Bs, 3], f32, kind="ExternalOutput")

    cshapes = dict(lhsT_w=[8, 128], bias4=[128, 1], Mh=[128, 128],
                   Mcomb2=[32, 128], chat2=[128, 1], ones32=[32, 1],
                   ones_bf=[128, 1], lhsT_fold=[128, 32], F2c2=[128, 3],
                   g2=[128, 1], b2=[128, 1], f2bias=[128, 1],
                   ident_bf=[128, 128], eps2=[128, 1], bog2=[128, 1])
    BF16_CONSTS = {"lhsT_w", "Mh", "ident_bf", "ones_bf"}
    cdram = {k: nc.dram_tensor(k, s, bf if k in BF16_CONSTS else f32,
                               kind="ExternalInput")
             for k, s in cshapes.items()}

    AF = mybir.ActivationFunctionType
    OP = mybir.AluOpType
    AX = mybir.AxisListType

    with tile.TileContext(nc) as tc:
        from contextlib import ExitStack
        ctx = ExitStack()
        with ctx:
            consts = ctx.enter_context(tc.tile_pool(name="consts", bufs=1))
            C = {}
            CRIT = ("ident_bf", "lhsT_w", "bias4", "ones_bf")
            for k, s in cshapes.items():
                C[k] = consts.tile(s, bf if k in BF16_CONSTS else f32,
                                   tag=k, name=f"c_{k}")
                if k in CRIT:
                    nc.sync.dma_start(C[k][:], cdram[k].ap())

            # ACT table preload (sqrt set includes relu/identity/copy/square)
            warm = consts.tile([128, 1], f32, tag="warm")
            nc.scalar.activation(warm[:], C["chat2"][:], AF.Sqrt,
                                 bias=C["eps2"][:], scale=0.0)

            f4pool = ctx.enter_context(tc.tile_pool(name="f4", bufs=1))
            f4 = f4pool.tile([128, Wtot], bf, tag="f4")

            # ---------------- PASS 1: x -> f4 -> gram (fused) ----------------
            import contextlib
            gram_scope = contextlib.ExitStack()
            gramps = gram_scope.enter_context(
                tc.tile_pool(name="p1_gram", bufs=1, space="PSUM"))
            pay4 = gramps.tile([128, 133], f32, tag="pay4")
            n_f4 = 0
            n_mm = 0
            N_MM = NG * (WG // 128) * 5   # total gram-psum matmuls
            with tc.tile_pool(name="p1_xsb", bufs=1) as pxsb, \
                 tc.tile_pool(name="p1_xpm", bufs=3) as pxpm, \
                 tc.tile_pool(name="p1_xts", bufs=NG) as pxts, \
                 tc.tile_pool(name="p1_xp", bufs=NG) as pxp, \
                 tc.tile_pool(name="p1_ft", bufs=2) as pft, \
                 tc.tile_pool(name="p1_ps_xt", bufs=2, space="PSUM") as psxt, \
                 tc.tile_pool(name="p1_ps_fp", bufs=2, space="PSUM") as psfp:
                x_sb = pxsb.tile([128, 3 * NB], f32, tag="x_sb")
                xsrc = x.ap().rearrange("(p i) c -> p (i c)", p=P)
                NLD = 8
                for ld in range(NLD):
                    CH = 3 * NB // NLD
                    nc.sync.dma_start(x_sb[:, CH * ld:CH * (ld + 1)],
                                      xsrc[:, CH * ld:CH * (ld + 1)])
                for k in cshapes:
                    if k not in CRIT:
                        nc.sync.dma_start(C[k][:], cdram[k].ap())

                # ---- phase A: x permute/transpose/gather for all groups ----
                xps = []
                for G in range(NG):
                    # permute x into transpose-block order:
                    # xpm col = beta*128 + 16*(4*cp+a) + 8*cci + g
                    # source x_sb col = (i' * 3 + c), i' = beta*64+cci*32+g*4+a
                    xpm = pxpm.tile([128, 512], bf, tag="xpm")
                    xv = x_sb[:, 768 * G:768 * (G + 1)].rearrange(
                        "p (beta cci g a c) -> p c beta a cci g",
                        beta=4, cci=2, g=8, a=4, c=3)
                    dv = xpm[:].rearrange(
                        "p (beta cp a cci g) -> p cp beta a cci g",
                        beta=4, cp=2, a=4, cci=2, g=8)
                    for cp in range(2):
                        nc.vector.tensor_copy(dv[:, cp], xv[:, 2 * cp])
                    xT_ps = psxt.tile([128, 512], bf, tag="xT")
                    for blk in range(4):
                        nc.tensor.transpose(
                            xT_ps[:, 128 * blk:128 * (blk + 1)],
                            xpm[:, 128 * blk:128 * (blk + 1)], C["ident_bf"][:])
                    xTs = pxts.tile([128, 512], bf, tag="xTs")
                    nc.vector.tensor_copy(xTs[:], xT_ps[:])
                    xp = pxp.tile([8, WG], bf, tag="xp")
                    # one gather per group: src partition (q*16+s) -> dst
                    # [q, s*512 + c]; both sides walk elements in the same order
                    nc.sync.dma_start(
                        xp[:].rearrange("q (s c) -> q s c", s=16),
                        xTs[:])
                    xps.append(xp)

                # ---- phase B: f_pre -> f4 -> fT -> gram ----
                for G in range(NG):
                    xp = xps[G]
                    for k in range(8):
                        fp_ps = psfp.tile([128, 1024], f32, tag="fpre")
                        for q2 in range(2):
                            wloc = k * 1024 + q2 * 512
                            nc.tensor.matmul(
                                fp_ps[:, 512 * q2:512 * (q2 + 1)],
                                C["lhsT_w"][:], xp[:, wloc:wloc + 512],
                                start=True, stop=True)
                        wglob = G * WG + k * 1024
                        if _bres(n_f4, p1_act, 32):
                            nc.scalar.activation(
                                f4[:, wglob:wglob + 1024], fp_ps[:],
                                AF.Relu, bias=C["bias4"][:])
                        else:
                            nc.vector.tensor_scalar(
                                f4[:, wglob:wglob + 1024], fp_ps[:],
                                C["bias4"][:], 0.0, op0=OP.add, op1=OP.max)
                        n_f4 += 1
                        # quarter-group fT transpose + gram as soon as the 2
                        # f4 tiles feeding it are written
                        if k % 2 == 1:
                            qt = k // 2
                            fT = pft.tile([128, WG // 4], bf, tag="fT")
                            lo = G * WG + qt * (WG // 4)
                            nc.sync.dma_start(
                                fT[:].rearrange("p (b r) -> p b r", r=128),
                                f4[:, lo:lo + WG // 4], transpose=True)
                            fTv = fT[:].rearrange("p (b r) -> p b r", r=128)
                            for b in range(WG // 512):
                                for al in range(4):
                                    nc.tensor.matmul(
                                        pay4[:, 32 * al:32 * al + 32],
                                        fTv[:, b, :],
                                        fTv[:, b, 32 * al:32 * al + 32],
                                        start=(n_mm == 0), stop=False)
                                    n_mm += 1
                                nc.tensor.matmul(
                                    pay4[:, 128:129],
                                    fTv[:, b, :], C["ones_bf"][:],
                                    start=(n_mm == 0), stop=(n_mm == N_MM - 1))
                                n_mm += 1

            # ---------------- mid: stats + AllReduce ----------------
            statpool = ctx.enter_context(tc.tile_pool(name="stat", bufs=1))
            with gram_scope:
                pay4_sb = statpool.tile([128, 133], f32, tag="pay4_sb")
                nc.vector.tensor_copy(pay4_sb[:], pay4[:])

            with tc.tile_pool(name="mid_sb", bufs=1) as midsb, \
                 tc.tile_pool(name="mid_ps", bufs=1, space="PSUM") as midps, \
                 tc.tile_pool(name="mid_dram", bufs=1, space="DRAM") as middr:
                Gcols = midsb.tile([32, 128], f32, tag="Gcols")
                for al in range(4):
                    nc.sync.dma_start(
                        Gcols[0:32, 32 * al:32 * al + 32],
                        pay4_sb[32 * al:32 * al + 32, 32 * al:32 * al + 32])
                pay = midsb.tile([32, 33], f32, tag="pay")
                nc.vector.tensor_reduce(
                    pay[:, 0:32],
                    Gcols[:].rearrange("p (a j) -> p j a", a=4),
                    axis=AX.X, op=OP.add)
                sf32_ps = midps.tile([32, 2], f32, tag="sf32ps")
                nc.tensor.matmul(sf32_ps[:, 0:1], C["lhsT_fold"][:],
                                 pay4_sb[:, 128:129], start=True, stop=True)
                nc.scalar.copy(pay[:, 32:33], sf32_ps[:, 0:1])

                ar_in = middr.tile([32, 33], f32)
                ar_out = middr.tile([32, 33], f32, addr_space="Shared")
                nc.sync.dma_start(ar_in[:], pay[:])
                if collective:
                    nc.gpsimd.collective_compute(
                        "AllReduce", OP.add,
                        replica_groups=[list(range(n_cores))],
                        ins=[ar_in.opt()], outs=[ar_out.opt()])
                else:
                    nc.gpsimd.dma_start(ar_out[:], ar_in[:])
                payg = midsb.tile([32, 33], f32, tag="payg")
                nc.sync.dma_start(payg[:], ar_out[:])
                G32 = payg[:, 0:32]
                SF32 = payg[:, 32:33]

                # ---- stats math on [128,1] replicated vectors (both
                # 64-halves carry the same data so bos IS b_rep directly) ----
                Bf = float(Bs * n_cores)
                mt_ps = midps.tile([128, 2], f32, tag="mtps")
                nc.tensor.matmul(mt_ps[:, 0:1], C["Mcomb2"][:], SF32,
                                 start=True, stop=True)
                v1_ps = midps.tile([32, 128], f32, tag="v1ps")
                nc.tensor.matmul(v1_ps[:], G32, C["Mcomb2"][:],
                                 start=True, stop=True)
                w1 = midsb.tile([32, 128], f32, tag="w1")
                nc.vector.tensor_mul(w1[:], v1_ps[:], C["Mcomb2"][:])
                mgm_ps = midps.tile([128, 2], f32, tag="mgmps")
                nc.tensor.matmul(mgm_ps[:, 0:1], w1[:], C["ones32"][:],
                                 start=True, stop=True)

                sv = midsb.tile([128, 10], f32, tag="sv")
                q_ = sv[:, 0:1]; mu = sv[:, 1:2]; t2 = sv[:, 2:3]
                t3 = sv[:, 3:4]; e2 = sv[:, 4:5]; var = sv[:, 5:6]
                sd = sv[:, 6:7]; inv = sv[:, 7:8]; s_c = sv[:, 8:9]
                bi = sv[:, 9:10]
                nc.scalar.activation(q_, mt_ps[:, 0:1], AF.Copy, scale=1.0 / Bf)
                nc.scalar.activation(mu, mt_ps[:, 0:1], AF.Identity,
                                     bias=C["chat2"][:], scale=1.0 / Bf)
                nc.vector.tensor_add(t2, q_, mu)
                nc.vector.tensor_mul(t3, C["chat2"][:], t2)
                nc.scalar.activation(e2, mgm_ps[:, 0:1], AF.Identity,
                                     bias=t3, scale=1.0 / Bf)
                nc.vector.tensor_mul(var, mu, mu)
                nc.vector.tensor_sub(var, e2, var)
                nc.scalar.activation(sd, var, AF.Sqrt, bias=C["eps2"][:])
                nc.vector.reciprocal(inv, sd)
                nc.vector.tensor_mul(s_c, C["g2"][:], inv)
                # bias = s*(chat - mu) + b
                nc.vector.tensor_sub(bi, C["chat2"][:], mu)
                nc.vector.tensor_mul(bi, s_c, bi)
                nc.vector.tensor_add(bi, bi, C["b2"][:])

                s_rep = statpool.tile([128, 1], f32, tag="s_rep")
                b_rep = statpool.tile([128, 1], f32, tag="b_rep")
                f2sb = statpool.tile([128, 32], bf, tag="f2sb")
                nc.vector.memset(f2sb[:], 0.0)
                if fast_bn:
                    # rh = relu(h + bos), bos = (chat-mu) + (b/g)*sd ; s folded
                    # into the f2 weights so scale never touches the hot loop
                    nc.vector.tensor_sub(b_rep[:], C["chat2"][:], mu)
                    t7 = sv[:, 3:4]
                    nc.vector.tensor_mul(t7, C["bog2"][:], sd)
                    nc.vector.tensor_add(b_rep[:], b_rep[:], t7)
                    F2cs = midsb.tile([128, 3], bf, tag="F2cs")
                    nc.vector.tensor_scalar(F2cs[:], C["F2c2"][:], s_c, None,
                                            op0=OP.mult)
                    for hh in range(2):
                        nc.vector.tensor_copy(
                            f2sb[64 * hh:64 * hh + 64, 16 * hh:16 * hh + 3],
                            F2cs[64 * hh:64 * hh + 64, :])
                else:
                    nc.vector.tensor_copy(s_rep[:], s_c)
                    nc.vector.tensor_copy(b_rep[:], bi)
                    for hh in range(2):
                        nc.vector.tensor_copy(
                            f2sb[64 * hh:64 * hh + 64, 16 * hh:16 * hh + 3],
                            C["F2c2"][64 * hh:64 * hh + 64, :])

            # ---------------- PASS 2 ----------------
            ydst = y.ap().rearrange("(p i) m -> p (i m)", p=P)
            n_rh = 0
            n_yst = 0
            n_yc = 0
            with tc.tile_pool(name="p3_rh", bufs=6) as p3rh, \
                 tc.tile_pool(name="p3_ybat", bufs=2) as p3yb, \
                 tc.tile_pool(name="p3_yt", bufs=2) as p3yt, \
                 tc.tile_pool(name="p3_yc", bufs=3) as p3yc, \
                 tc.tile_pool(name="p3_hps", bufs=3, space="PSUM") as hpsp, \
                 tc.tile_pool(name="p3_yps", bufs=2, space="PSUM") as ypsp:
                for G in range(NG):
                    ybat = p3yb.tile([128, 4096], bf, tag="ybat")
                    for jj in range(8):
                        py_ps = ypsp.tile([128, 512], f32, tag="pyps")
                        for wi in range(2):
                            wt = G * 16 + 2 * jj + wi
                            h_ps = hpsp.tile([128, 1024], f32, tag="hps")
                            for sg in range(2):
                                nc.tensor.matmul(
                                    h_ps[:, 512 * sg:512 * (sg + 1)],
                                    C["Mh"][64 * sg:64 * (sg + 1), :],
                                    f4[64 * sg:64 * sg + 64, 512 * wt:512 * (wt + 1)],
                                    start=True, stop=True)
                            rh = p3rh.tile([128, 1024], bf, tag="rh")
                            if fast_bn:
                                if _bres(n_rh, rh_act, 64):
                                    nc.scalar.activation(rh[:], h_ps[:], AF.Relu,
                                                         bias=b_rep[:])
                                else:
                                    nc.vector.tensor_scalar(
                                        rh[:], h_ps[:], b_rep[:], 0.0,
                                        op0=OP.add, op1=OP.max)
                            else:
                                nc.scalar.activation(rh[:], h_ps[:], AF.Relu,
                                                     bias=b_rep[:], scale=s_rep[:])
                            n_rh += 1
                            for sg in range(2):
                                t = 2 * wi + sg
                                nc.tensor.matmul(
                                    py_ps[32 * t:32 * t + 32, :], f2sb[:],
                                    rh[:, 512 * sg:512 * (sg + 1)],
                                    start=True, stop=True,
                                    tile_position=(0, 32 * t))
                        if _bres(n_yst, yst_act, 32):
                            nc.scalar.activation(
                                ybat[:, 512 * jj:512 * (jj + 1)], py_ps[:],
                                AF.Identity, bias=C["f2bias"][:])
                        else:
                            nc.vector.tensor_scalar(
                                ybat[:, 512 * jj:512 * (jj + 1)], py_ps[:],
                                C["f2bias"][:], None, op0=OP.add)
                        n_yst += 1
                    yt = p3yt.tile([128, 4096], bf, tag="yt")
                    for hf in range(2):
                        nc.sync.dma_start(
                            yt[:, 2048 * hf:2048 * (hf + 1)].rearrange(
                                "p (b r) -> p b r", r=128),
                            ybat[:, 2048 * hf:2048 * (hf + 1)], transpose=True)
                    # yc[p, 3*i' + m], i' = 64*beta + 32*cci + 4*g + 2*sig + sg
                    yc = p3yc.tile([128, 768], f32, tag="yc")
                    ytv = yt[:].rearrange("p (k r) -> p k r", r=128)
                    for jj in range(8):
                        for wi in range(2):
                            v = 2 * jj + wi
                            cci, gg = v // 8, v % 8
                            # src dims (beta, af=(sig,sg), m): r = 64*wi+16*af+m
                            src = ytv[:, 4 * jj:4 * jj + 4, :].rearrange(
                                "p beta (wihalf af m) -> p wihalf beta af m",
                                wihalf=2, af=4, m=16)[:, wi, :, :, 0:3]
                            dst = yc[:].rearrange(
                                "p (beta cci g af m) -> p cci g beta af m",
                                beta=4, cci=2, g=8, af=4, m=3)[:, cci, gg]
                            if _bres(n_yc, yc_pool, 4):
                                nc.gpsimd.tensor_copy(dst, src)
                            else:
                                nc.vector.tensor_copy(dst, src)
                            n_yc += 1
                    nc.sync.dma_start(
                        ydst[:, 768 * G:768 * (G + 1)], yc[:])

    nc.compile()
    return nc


def _get_nc(Bs, n_cores, fast_bn):
    key = (Bs, n_cores, fast_bn)
    if key not in _CACHE:
        _CACHE[key] = _build(Bs, n_cores, fast_bn=fast_bn)
    return _CACHE[key]


def kernel(**inputs):
    from concourse.bass_utils import run_bass_kernel_spmd

    x_full = np.ascontiguousarray(np.asarray(inputs["x"], np.float32))
    B = x_full.shape[0]
    assert B % N_CORES == 0
    Bs = B // N_CORES
    w = {k: np.asarray(v) for k, v in inputs.items() if k != "x"}
    consts = _derive_host_constants(w)

    fast_bn = bool(np.all(np.asarray(w["bn_g"]) > 0))
    nc = _get_nc(Bs, N_CORES, fast_bn)
    in_maps = []
    for c in range(N_CORES):
        m = {"x": np.ascontiguousarray(x_full[c * Bs:(c + 1) * Bs])}
        m.update(consts)
        in_maps.append(m)
    res = run_bass_kernel_spmd(nc, in_maps, core_ids=list(range(N_CORES)))
    return np.concatenate([res.results[c]["y"] for c in range(N_CORES)], axis=0)
